# revision 4
# baseline (speedup 1.0000x reference)
"""Trainium2 Bass kernel for nn_CLUBCategorical (CLUB categorical loss).

Reference computation:
    h      = relu(x @ W1 + b1)              [N, H]
    logits = h @ W2 + b2                    [N, Y]
    logp   = log_softmax(logits, -1)        [N, Y]
    out[i] = logp[i, y_i] - mean_j logp[i, y_j]

The log-softmax normalizer cancels between the positive and negative
terms. With c[y] = histogram(y_idx) (global), w2c = (W2 @ c)/N:

    out[i] = h[i,:] @ (W2[:, y_i] - w2c) + (b2[y_i] - (b2 @ c)/N)
           = h[i,:] @ A[:, i] + g[i]

A is gathered on the HOST (it knows y), so the device only computes
phase-1 (h = relu(x@W1+b1), 64 matmuls) plus a cheap product-reduce:

    per m-chunk (128 hidden dims):  prod_m = hT_m * A_m      (DVE, bf16)
    acc = sum_{m<7} prod_m                                    (DVE chain)
    rgroup 0: out[1,512] = ones^T @ acc (+) ones^T @ prod_7   (2 matmuls,
                              PSUM-accumulated, mid-flight, then copied)
    rgroup 1: raw `acc1` ships EARLY plus the last chunk's product `p7`
              the moment it exists (its relu split across Scalar and
              DVE); the 128-partition reduce happens on the HOST.

g is added on the host during unsharding.

Hardware model (from neuron-profile traces of the previous builds):
- HAM clock gate: PE at 1.2GHz until ~3.4-5us of SUSTAINED matmul
  activity; an idle hole resets the streak (a 2.4us hole cost ~6us of
  cold execution in one build).  So the warmup spins must bridge the
  DMA lead-in exactly, with mini-spins over any known data gap.
- DMA: each transfer's completion semaphore fires ~1.3-2us after its
  last byte (HBM receipt round trip).  Queues (rings) round-robin at
  packet granularity, so concurrent streams split bandwidth; the
  critical prefix (xt0-k0 + w1-m0k0) is made SMALL and runs against
  only light contention.  The Tile scheduler keeps per-engine program
  order only among equally-ready instructions: a dep-blocked "gate"
  gets bypassed, so every deferred A config carries its OWN data dep
  (a dummy gpsimd write into that A tile, reading an xt1b-gated
  slice).  The sync ring's A-g0 configs are all ready at t=0 and so
  stay FIFO behind the xt stream without any gate.
- DMA configs block their issuing queue at ring depth 4; Scalar
  (which runs the 16 ReLUs from ~10.7us) carries exactly 4 input
  configs, issued before the first relu.

Sharding: data-parallel over N; each of 8 cores takes 1024 rows and the
full W1 plus its own gathered A block. No collectives.
"""

import numpy as np

N, X_DIM, Y_DIM, HIDDEN = 8192, 512, 512, 1024
N_CORES = 8
N_LOC = N // N_CORES          # 1024 rows per core
KX = X_DIM // 128             # 4  k-chunks (contraction), phase 1
KH = HIDDEN // 128            # 8  hidden chunks
RG = N_LOC // 512             # 2  row groups of 512

N_WU = 6                      # warmup matmuls (bridge until xt0k0 lands)
N_WU_MID = 3                  # 256-wide spins bridging xt0k1 -> xt0b

_NC_CACHE = {}


def _build(nc_cls, mybir, tile):
    mdt = mybir.dt
    f32 = mdt.float32
    F32R = mdt.float32r
    BF16 = mdt.bfloat16
    AF = mybir.ActivationFunctionType
    OP = mybir.AluOpType

    nc = nc_cls("TRN2", target_bir_lowering=False, debug=False,
                num_devices=N_CORES)

    # --- dram tensors (bf16 device layouts) ---
    # xt0 in three pieces (k0 | k1 | k23); xt1 in halves.
    xt0D = [nc.dram_tensor("xt0k0", [128, 512], BF16, kind="ExternalInput"),
            nc.dram_tensor("xt0k1", [128, 512], BF16, kind="ExternalInput"),
            nc.dram_tensor("xt0b", [128, 1024], BF16, kind="ExternalInput")]
    xt1D = [nc.dram_tensor(f"xt1{h}", [128, 1024], BF16,
                           kind="ExternalInput") for h in "ab"]
    # w1 split by consumption: m0k0 | m0k123 | m1-3 | m4-7
    # layout w1[p, m*512 + k*128 + c] = W1[k*128+p, m*128+c]
    w1D = [nc.dram_tensor("w1a0", [128, 128], BF16, kind="ExternalInput"),
           nc.dram_tensor("w1a1", [128, 384], BF16, kind="ExternalInput"),
           nc.dram_tensor("w1b", [128, 1536], BF16, kind="ExternalInput"),
           nc.dram_tensor("w1c", [128, 2048], BF16, kind="ExternalInput")]
    # am{m}{g}: [128, 512]  a[p, r] = W2m[m*128+p, y[g*512+r]]
    aD = [[nc.dram_tensor(f"am{m}g{g}", [128, 512], BF16,
                          kind="ExternalInput") for g in range(RG)]
          for m in range(KH)]
    # b1c: [128, 8]  b1c[p, m] = b1[m*128+p]
    b1D = nc.dram_tensor("b1c", [128, KH], f32, kind="ExternalInput")
    out = nc.dram_tensor("out", [1, N_LOC], f32, kind="ExternalOutput")
    acc1D = nc.dram_tensor("acc1", [128, 512], BF16, kind="ExternalOutput")
    p7D = nc.dram_tensor("p7", [128, 512], BF16, kind="ExternalOutput")

    with tile.TileContext(nc) as tc:
        with (
            tc.tile_pool(name="wgt", bufs=1) as wgt,
            tc.tile_pool(name="hp", bufs=1) as hp,
            tc.tile_pool(name="pr", bufs=1) as pr,
            tc.tile_pool(name="ps", bufs=1, space="PSUM") as ps,
        ):
            # --- on-chip constants.  wu_f memset runs on gpsimd FIRST so
            # the 512-wide PE spins can start right at body start. ---
            wu_f = wgt.tile([128, 512], f32, tag="wu")
            nc.gpsimd.memset(wu_f[:], 1.0)
            wu_src = wu_f.bitcast(F32R)
            ones_f = wgt.tile([128, 1], f32, tag="onesf")
            nc.vector.memset(ones_f[:], 1.0)
            ones_sb = wgt.tile([128, 1], BF16, tag="ones")
            nc.vector.tensor_copy(ones_sb[:], ones_f[:])

            b1_sb = wgt.tile([128, KH], f32, tag="b1")
            xt_sb = [wgt.tile([128, KX * 512], BF16, tag=f"xt{n}",
                              name=f"xt{n}") for n in range(RG)]
            w1_sb = wgt.tile([128, KH * 512], BF16, tag="w1")
            a_sb = [wgt.tile([128, N_LOC], BF16, tag=f"a{m}",
                             name=f"a{m}") for m in range(KH)]

            # --- DMA.  scalar: w1 stream (4 configs, done before relus).
            # sync: xt stream then A-g0 halves (ring FIFO keeps order).
            # gpsimd: b1 early; A-g1 halves behind per-config dummies. ---
            nc.scalar.dma_start(w1_sb[:, 0:128], w1D[0].ap())
            nc.sync.dma_start(xt_sb[0][:, 0:512], xt0D[0].ap())
            nc.gpsimd.dma_start(b1_sb[:], b1D.ap())   # SWDGE: 32B/partition
            nc.scalar.dma_start(w1_sb[:, 128:512], w1D[1].ap())
            nc.sync.dma_start(xt_sb[0][:, 512:1024], xt0D[1].ap())
            nc.sync.dma_start(xt_sb[0][:, 1024:2048], xt0D[2].ap())
            nc.scalar.dma_start(w1_sb[:, 512:2048], w1D[2].ap())
            nc.scalar.dma_start(w1_sb[:, 2048:4096], w1D[3].ap())
            nc.sync.dma_start(xt_sb[1][:, 0:1024], xt1D[0].ap())
            nc.sync.dma_start(xt_sb[1][:, 1024:2048], xt1D[1].ap())
            for m in range(KH):
                nc.sync.dma_start(a_sb[m][:, 0:512], aD[m][0].ap())
            # A-g1: each config gets a LIVE WAW dep (dummy writes one cell
            # of the target tile, reading an xt1b-backed slice) so the
            # scheduler cannot hoist it into the critical DMA prefix.
            for m in range(KH):
                nc.gpsimd.tensor_scalar_mul(
                    a_sb[m][:, 512:513], xt_sb[1][:, 2047:2048], 0.0)
                nc.gpsimd.dma_start(a_sb[m][:, 512:1024], aD[m][1].ap())

            # --- PE warmup: tiny const-ap matmuls at body start, then
            # 512-wide spins until xt0k0 lands. ---
            cap = nc.const_aps.aps[(f32, 1.0)]
            wupre = ps.tile([1, 1], f32, tag="psum", bufs=6, name="wupre")
            for _ in range(8):
                nc.tensor.matmul(wupre[:], cap, cap, start=True, stop=True)
            wu = ps.tile([128, 512], f32, tag="psum", bufs=6, name="wu")
            for _ in range(N_WU):
                nc.tensor.matmul(wu[:], wu_src[:, 0:128], wu_src[:],
                                 start=True, stop=True)

            hT = [hp.tile([128, N_LOC], BF16, tag=f"h{m}", name=f"h{m}")
                  for m in range(KH)]
            prod = {}
            acc = {}

            def a_slice(n, m):
                return a_sb[m][:, n * 512:(n + 1) * 512]

            psums = {}

            def p1_mms(n, m, k0, k1):
                if (n, m) not in psums:
                    psums[(n, m)] = ps.tile([128, 512], f32, tag="psum",
                                            bufs=6, name=f"p1_{n}_{m}")
                psum = psums[(n, m)]
                for k in range(k0, k1):
                    nc.tensor.matmul(
                        psum[:],
                        w1_sb[:, m * 512 + k * 128: m * 512 + (k + 1) * 128],
                        xt_sb[n][:, k * 512:(k + 1) * 512],
                        start=(k == 0), stop=(k == KX - 1))

            def p1_act(n, m):
                nc.scalar.activation(
                    hT[m][:, n * 512:(n + 1) * 512], psums[(n, m)][:],
                    AF.Relu, bias=b1_sb[:, m:m + 1])

            def phase1(n, m):
                p1_mms(n, m, 0, KX)
                p1_act(n, m)

            def product(n, m):
                p = pr.tile([128, 512], BF16, tag=f"pr{n}_{m}",
                            name=f"pr{n}_{m}")
                nc.vector.tensor_tensor(
                    p[:], hT[m][:, n * 512:(n + 1) * 512],
                    a_slice(n, m), OP.mult)
                prod[(n, m)] = p
                if m == 0:
                    acc[n] = p
                elif m < KH - 1:   # m7 handled separately (matmul / p7D)
                    a2 = pr.tile([128, 512], BF16, tag=f"ac{n}_{m}",
                                 name=f"ac{n}_{m}")
                    nc.vector.tensor_tensor(a2[:], acc[n][:], p[:], OP.add)
                    acc[n] = a2

            pout = {}

            def reduce_pre(n):     # ones^T @ acc(m0..m6) -> pout[n]
                po = ps.tile([1, 512], f32, tag=f"po{n}", bufs=1,
                             name=f"po{n}")
                nc.tensor.matmul(po[:], ones_sb[:], acc[n][:],
                                 start=True, stop=False)
                pout[n] = po

            o_sb = wgt.tile([1, N_LOC], f32, tag="o")

            def reduce_fin(n):     # += ones^T @ prod_7, copy out, DMA
                nc.tensor.matmul(pout[n][:], ones_sb[:], prod[(n, KH - 1)][:],
                                 start=False, stop=True)
                # psum -> sbuf on the (idle) Scalar engine: `copy` lives in
                # the same act table as relu, so no table reload.
                nc.scalar.activation(o_sb[:, n * 512:(n + 1) * 512],
                                     pout[n][:], AF.Copy)
                eng = nc.sync if n == 0 else nc.scalar
                eng.dma_start(out.ap()[:, n * 512:(n + 1) * 512],
                              o_sb[:, n * 512:(n + 1) * 512])

            # --- schedule. Chunk (0, m0) runs k0 then k1 the moment each
            # xt piece lands; mini-spins bridge the known xt0b gap so the
            # PE streak never breaks. ---
            p1_mms(0, 0, 0, 2)
            for _ in range(N_WU_MID):
                nc.tensor.matmul(wu[:, 0:256], wu_src[:, 0:128],
                                 wu_src[:, 0:256], start=True, stop=True)
            p1_mms(0, 0, 2, KX)
            p1_act(0, 0)
            product(0, 0)
            for m in range(1, KH):
                phase1(0, m)
                product(0, m)
            m7 = KH - 1
            czero = nc.const_aps.aps[(f32, 0.0)]
            for m in range(KH):
                phase1(1, m) if m < m7 else p1_mms(1, m, 0, KX)
                if m == 3:
                    reduce_pre(0)
                elif m == 4:
                    reduce_fin(0)
                if m < m7:
                    product(1, m)
                else:
                    # tail: split the last relu across Scalar and DVE so
                    # the 0.7us activation no longer serializes the chain
                    nc.scalar.activation(
                        hT[m7][:, 512:768], psums[(1, m7)][:, 0:256],
                        AF.Relu, bias=b1_sb[:, m7:m7 + 1])
                    nc.vector.tensor_scalar(
                        hT[m7][:, 768:1024], psums[(1, m7)][:, 256:512],
                        b1_sb[:, m7:m7 + 1], czero, OP.add, OP.max)
                    p = pr.tile([128, 512], BF16, tag="pr1_7s",
                                name="pr1_7s")
                    nc.vector.tensor_tensor(
                        p[:, 256:512], hT[m7][:, 768:1024],
                        a_sb[m7][:, 768:1024], OP.mult)
                    nc.vector.tensor_tensor(
                        p[:, 0:256], hT[m7][:, 512:768],
                        a_sb[m7][:, 512:768], OP.mult)
                    prod[(1, m7)] = p
            nc.sync.dma_start(acc1D.ap(), acc[1][:])
            nc.scalar.dma_start(p7D.ap(), prod[(1, KH - 1)][:])

    nc.compile()
    return nc


def _get_nc():
    if "nc" not in _NC_CACHE:
        import concourse.bacc as bacc
        import concourse.mybir as mybir
        from concourse import tile
        _NC_CACHE["nc"] = _build(bacc.Bacc, mybir, tile)
    return _NC_CACHE["nc"]


def kernel(x_samples, y_idx, W1, b1, W2, b2):
    import ml_dtypes
    from concourse.bass_utils import run_bass_kernel_spmd

    bf16 = ml_dtypes.bfloat16
    x = np.ascontiguousarray(np.asarray(x_samples, dtype=np.float32))
    y = np.asarray(y_idx).astype(np.int64).reshape(-1)
    W1 = np.ascontiguousarray(np.asarray(W1, dtype=np.float32))
    b1 = np.asarray(b1, dtype=np.float32).reshape(-1)
    W2 = np.ascontiguousarray(np.asarray(W2, dtype=np.float32))
    b2 = np.asarray(b2, dtype=np.float32).reshape(-1)

    # global label histogram; fold normalizer-free negative term + bias
    c = np.bincount(y, minlength=Y_DIM).astype(np.float32)
    w2c = (W2 @ c) / np.float32(N)                                # [H]
    beta = np.float32(b2 @ c) / np.float32(N)
    g_full = (b2[y] - beta).astype(np.float32)                    # [N]

    # device layouts
    # w1_dev[m][p, k*128+c] = W1[k*128+p, m*128+c]
    w1_dev = np.ascontiguousarray(
        W1.reshape(KX, 128, KH, 128).transpose(2, 1, 0, 3)
        .reshape(KH, 128, KX * 128)).astype(bf16)
    w1_flat = np.ascontiguousarray(
        w1_dev.transpose(1, 0, 2).reshape(128, KH * 512))
    b1c = np.ascontiguousarray(b1.reshape(KH, 128).T)             # [128, 8]
    W2m = W2 - w2c[:, None]                                       # [H, Y]

    in_maps = []
    for mcore in range(N_CORES):
        sl = slice(mcore * N_LOC, (mcore + 1) * N_LOC)
        # xt_dev[n][p, k*512+r] = x[base + n*512+r, k*128+p]
        xt_dev = np.ascontiguousarray(
            x[sl].reshape(RG, 512, KX, 128).transpose(0, 3, 2, 1)
            .reshape(RG, 128, KX * 512)).astype(bf16)
        # a_dev[m][p, r] = W2m[m*128+p, y[base+r]]
        a_dev = W2m[:, y[sl]].reshape(KH, 128, N_LOC).astype(bf16)
        im = {
            "b1c": b1c,
            "w1a0": np.ascontiguousarray(w1_flat[:, 0:128]),
            "w1a1": np.ascontiguousarray(w1_flat[:, 128:512]),
            "w1b": np.ascontiguousarray(w1_flat[:, 512:2048]),
            "w1c": np.ascontiguousarray(w1_flat[:, 2048:4096]),
            "xt0k0": np.ascontiguousarray(xt_dev[0][:, 0:512]),
            "xt0k1": np.ascontiguousarray(xt_dev[0][:, 512:1024]),
            "xt0b": np.ascontiguousarray(xt_dev[0][:, 1024:2048]),
            "xt1a": np.ascontiguousarray(xt_dev[1][:, 0:1024]),
            "xt1b": np.ascontiguousarray(xt_dev[1][:, 1024:2048]),
        }
        for m in range(KH):
            for g in range(RG):
                im[f"am{m}g{g}"] = np.ascontiguousarray(
                    a_dev[m][:, g * 512:(g + 1) * 512])
        in_maps.append(im)

    nc = _get_nc()
    res = run_bass_kernel_spmd(nc, in_maps, core_ids=list(range(N_CORES)))
    parts = []
    for mc in range(N_CORES):
        r = res.results[mc]
        parts.append(np.asarray(r["out"]).reshape(-1)[0:512])
        # rgroup 1 ships its raw m0-6 product-accumulator (early, off the
        # tail) plus the last chunk's product separately; the partition
        # reduce happens here (identical math to the PSUM ones-matmul).
        parts.append(np.asarray(r["acc1"], dtype=np.float32).sum(axis=0)
                     + np.asarray(r["p7"], dtype=np.float32).sum(axis=0))
    dev = np.concatenate(parts)
    return (dev + g_full).astype(np.float32)


# revision 11
# speedup vs baseline: 1.0254x; 1.0254x over previous
"""Trainium2 Bass kernel for nn_CLUBCategorical (CLUB categorical loss).

Reference computation:
    h      = relu(x @ W1 + b1)              [N, H]
    logits = h @ W2 + b2                    [N, Y]
    logp   = log_softmax(logits, -1)        [N, Y]
    out[i] = logp[i, y_i] - mean_j logp[i, y_j]

The log-softmax normalizer cancels between the positive and negative
terms. With c[y] = histogram(y_idx) (global), w2c = (W2 @ c)/N:

    out[i] = h[i,:] @ (W2[:, y_i] - w2c) + (b2[y_i] - (b2 @ c)/N)
           = h[i,:] @ A[:, i] + g[i]

A is gathered on the HOST (it knows y), so the device only computes
phase-1 (h = relu(x@W1+b1), 64 matmuls) plus a cheap product-reduce:

    per m-chunk (128 hidden dims):  prod_m = hT_m * A_m      (DVE, bf16)
    acc = sum_{m<7} prod_m                                    (DVE chain)
    rgroup 0: out[1,512] = ones^T @ acc (+) ones^T @ prod_7   (2 matmuls,
                              PSUM-accumulated, mid-flight, then copied)
    rgroup 1: raw `acc1` ships EARLY plus the last chunk's product `p7`
              the moment it exists (its relu split across Scalar and
              DVE); the 128-partition reduce happens on the HOST.

g is added on the host during unsharding.

Hardware model (from neuron-profile traces of the previous builds):
- HAM clock gate: PE at 1.2GHz until ~3.4-5us of SUSTAINED matmul
  activity; an idle hole resets the streak (a 2.4us hole cost ~6us of
  cold execution in one build).  So the warmup spins must bridge the
  DMA lead-in exactly, with mini-spins over any known data gap.
- DMA: each transfer's completion semaphore fires ~1.3-2us after its
  last byte (HBM receipt round trip).  Queues (rings) round-robin at
  packet granularity, so concurrent streams split bandwidth; the
  critical prefix (xt0-k0 + w1-m0k0) is made SMALL and runs against
  only light contention.  The Tile scheduler keeps per-engine program
  order only among equally-ready instructions: a dep-blocked "gate"
  gets bypassed, so every deferred A config carries its OWN data dep
  (a dummy gpsimd write into that A tile, reading an xt1b-gated
  slice).  The sync ring's A-g0 configs are all ready at t=0 and so
  stay FIFO behind the xt stream without any gate.
- DMA configs block their issuing queue at ring depth 4; Scalar
  (which runs the 16 ReLUs from ~10.7us) carries exactly 4 input
  configs, issued before the first relu.

Sharding: data-parallel over N; each of 8 cores takes 1024 rows and the
full W1 plus its own gathered A block. No collectives.
"""

import numpy as np

N, X_DIM, Y_DIM, HIDDEN = 8192, 512, 512, 1024
N_CORES = 8
N_LOC = N // N_CORES          # 1024 rows per core
KX = X_DIM // 128             # 4  k-chunks (contraction), phase 1
KH = HIDDEN // 128            # 8  hidden chunks
RG = N_LOC // 512             # 2  row groups of 512

N_WU = 6                      # warmup matmuls (bridge until xt0k0 lands)
N_WU_MID = 4                  # 256-wide spins bridging xt0k1 -> xt0b

_NC_CACHE = {}


def _build(nc_cls, mybir, tile):
    mdt = mybir.dt
    f32 = mdt.float32
    F32R = mdt.float32r
    BF16 = mdt.bfloat16
    AF = mybir.ActivationFunctionType
    OP = mybir.AluOpType

    nc = nc_cls("TRN2", target_bir_lowering=False, debug=False,
                num_devices=N_CORES)

    # --- dram tensors (bf16 device layouts) ---
    # xt0 in three pieces (k0 | k1 | k23); xt1 in halves.
    xt0D = [nc.dram_tensor("xt0k0", [128, 512], BF16, kind="ExternalInput"),
            nc.dram_tensor("xt0k1", [128, 512], BF16, kind="ExternalInput"),
            nc.dram_tensor("xt0b", [128, 1024], BF16, kind="ExternalInput")]
    xt1D = [nc.dram_tensor(f"xt1{h}", [128, 1024], BF16,
                           kind="ExternalInput") for h in "ab"]
    # w1 split per m-chunk (m0 further split k0 | k123): each piece is
    # consumed by exactly one chunk's 4 matmuls, so the stream can land
    # in deadline order at ~128KB granularity.
    # layout w1[p, m*512 + k*128 + c] = W1[k*128+p, m*128+c]
    w1D = [nc.dram_tensor("w1a0", [128, 128], BF16, kind="ExternalInput"),
           nc.dram_tensor("w1a1", [128, 384], BF16, kind="ExternalInput")]
    w1D += [nc.dram_tensor(f"w1m{m}", [128, 512], BF16,
                           kind="ExternalInput") for m in range(1, KH)]
    # am{m}{g}: [128, 512]  a[p, r] = W2m[m*128+p, y[g*512+r]]
    aD = [[nc.dram_tensor(f"am{m}g{g}", [128, 512], BF16,
                          kind="ExternalInput") for g in range(RG)]
          for m in range(KH)]
    # b1c: [128, 8]  b1c[p, m] = b1[m*128+p]
    b1D = nc.dram_tensor("b1c", [128, KH], f32, kind="ExternalInput")
    out = nc.dram_tensor("out", [1, N_LOC], f32, kind="ExternalOutput")
    acc1D = nc.dram_tensor("acc1", [128, 512], BF16, kind="ExternalOutput")
    p7D = nc.dram_tensor("p7", [128, 512], BF16, kind="ExternalOutput")

    with tile.TileContext(nc) as tc:
        with (
            tc.tile_pool(name="wgt", bufs=1) as wgt,
            tc.tile_pool(name="hp", bufs=1) as hp,
            tc.tile_pool(name="pr", bufs=1) as pr,
            tc.tile_pool(name="ps", bufs=1, space="PSUM") as ps,
        ):
            # --- on-chip constants.  wu_f memset runs on gpsimd FIRST so
            # the 512-wide PE spins can start right at body start. ---
            wu_f = wgt.tile([128, 512], f32, tag="wu")
            nc.gpsimd.memset(wu_f[:], 1.0)
            wu_src = wu_f.bitcast(F32R)
            ones_f = wgt.tile([128, 1], f32, tag="onesf")
            nc.vector.memset(ones_f[:], 1.0)
            ones_sb = wgt.tile([128, 1], BF16, tag="ones")
            nc.vector.tensor_copy(ones_sb[:], ones_f[:])

            b1_sb = wgt.tile([128, KH], f32, tag="b1")
            xt_sb = [wgt.tile([128, KX * 512], BF16, tag=f"xt{n}",
                              name=f"xt{n}") for n in range(RG)]
            w1_sb = wgt.tile([128, KH * 512], BF16, tag="w1")
            a_sb = [wgt.tile([128, N_LOC], BF16, tag=f"a{m}",
                             name=f"a{m}") for m in range(KH)]

            # --- DMA.  All transfers in GLOBAL DEADLINE ORDER, round-
            # robined across four rings.  Each ring's configs are all
            # ready at t=0, so per-ring priority order == program order
            # == ring FIFO: transfers drain in exactly this order, and
            # deferred transfers (A-g1) simply sit behind earlier ones on
            # the same deep ring -- no gates needed, no hoisting possible.
            # scalar and vector carry exactly 4 configs each (ring depth)
            # so their engines never block before the relus/products. ---
            def w1s(m):
                return w1_sb[:, m * 512:(m + 1) * 512]

            nc.scalar.dma_start(w1_sb[:, 0:128], w1D[0].ap())        # w1a0
            nc.sync.dma_start(xt_sb[0][:, 0:512], xt0D[0].ap())      # xt0k0
            nc.gpsimd.dma_start(b1_sb[:], b1D.ap())                  # b1
            nc.scalar.dma_start(w1_sb[:, 128:512], w1D[1].ap())      # w1a1
            nc.sync.dma_start(xt_sb[0][:, 512:1024], xt0D[1].ap())   # xt0k1
            nc.gpsimd.dma_start(xt_sb[0][:, 1024:2048], xt0D[2].ap())  # xt0b
            nc.scalar.dma_start(w1s(1), w1D[2].ap())                 # w1m1
            nc.sync.dma_start(a_sb[0][:, 0:512], aD[0][0].ap())      # A0g0
            nc.gpsimd.dma_start(w1s(2), w1D[3].ap())                 # w1m2
            nc.sync.dma_start(a_sb[1][:, 0:512], aD[1][0].ap())      # A1g0
            nc.scalar.dma_start(w1s(3), w1D[4].ap())                 # w1m3
            nc.gpsimd.dma_start(a_sb[2][:, 0:512], aD[2][0].ap())    # A2g0
            nc.sync.dma_start(w1s(4), w1D[5].ap())                   # w1m4
            nc.gpsimd.dma_start(a_sb[3][:, 0:512], aD[3][0].ap())    # A3g0
            nc.sync.dma_start(w1s(5), w1D[6].ap())                   # w1m5
            nc.gpsimd.dma_start(a_sb[4][:, 0:512], aD[4][0].ap())    # A4g0
            nc.sync.dma_start(w1s(6), w1D[7].ap())                   # w1m6
            nc.gpsimd.dma_start(a_sb[5][:, 0:512], aD[5][0].ap())    # A5g0
            nc.sync.dma_start(w1s(7), w1D[8].ap())                   # w1m7
            nc.gpsimd.dma_start(xt_sb[1][:, 0:1024], xt1D[0].ap())   # xt1a
            nc.sync.dma_start(xt_sb[1][:, 1024:2048], xt1D[1].ap())  # xt1b
            nc.gpsimd.dma_start(a_sb[6][:, 0:512], aD[6][0].ap())    # A6g0
            nc.sync.dma_start(a_sb[7][:, 0:512], aD[7][0].ap())      # A7g0
            for m in range(KH):
                eng = nc.gpsimd if m % 2 == 0 else nc.sync
                eng.dma_start(a_sb[m][:, 512:1024], aD[m][1].ap())   # A-g1

            # --- PE warmup: tiny const-ap matmuls at body start, then
            # 512-wide spins until xt0k0 lands. ---
            cap = nc.const_aps.aps[(f32, 1.0)]
            wupre = ps.tile([1, 1], f32, tag="psum", bufs=6, name="wupre")
            for _ in range(8):
                nc.tensor.matmul(wupre[:], cap, cap, start=True, stop=True)
            wu = ps.tile([128, 512], f32, tag="psum", bufs=6, name="wu")
            for _ in range(N_WU):
                nc.tensor.matmul(wu[:], wu_src[:, 0:128], wu_src[:],
                                 start=True, stop=True)

            hT = [hp.tile([128, N_LOC], BF16, tag=f"h{m}", name=f"h{m}")
                  for m in range(KH)]
            prod = {}
            acc = {}

            def a_slice(n, m):
                return a_sb[m][:, n * 512:(n + 1) * 512]

            psums = {}

            def p1_mms(n, m, k0, k1):
                if (n, m) not in psums:
                    psums[(n, m)] = ps.tile([128, 512], f32, tag="psum",
                                            bufs=6, name=f"p1_{n}_{m}")
                psum = psums[(n, m)]
                for k in range(k0, k1):
                    nc.tensor.matmul(
                        psum[:],
                        w1_sb[:, m * 512 + k * 128: m * 512 + (k + 1) * 128],
                        xt_sb[n][:, k * 512:(k + 1) * 512],
                        start=(k == 0), stop=(k == KX - 1))

            def p1_act(n, m):
                nc.scalar.activation(
                    hT[m][:, n * 512:(n + 1) * 512], psums[(n, m)][:],
                    AF.Relu, bias=b1_sb[:, m:m + 1])

            def phase1(n, m):
                p1_mms(n, m, 0, KX)
                p1_act(n, m)

            def product(n, m):
                p = pr.tile([128, 512], BF16, tag=f"pr{n}_{m}",
                            name=f"pr{n}_{m}")
                nc.vector.tensor_tensor(
                    p[:], hT[m][:, n * 512:(n + 1) * 512],
                    a_slice(n, m), OP.mult)
                prod[(n, m)] = p
                if m == 0:
                    acc[n] = p
                elif m < KH - 1:   # m7 handled separately (matmul / p7D)
                    a2 = pr.tile([128, 512], BF16, tag=f"ac{n}_{m}",
                                 name=f"ac{n}_{m}")
                    nc.vector.tensor_tensor(a2[:], acc[n][:], p[:], OP.add)
                    acc[n] = a2

            pout = {}

            def reduce_pre(n):     # ones^T @ acc(m0..m6) -> pout[n]
                po = ps.tile([1, 512], f32, tag=f"po{n}", bufs=1,
                             name=f"po{n}")
                nc.tensor.matmul(po[:], ones_sb[:], acc[n][:],
                                 start=True, stop=False)
                pout[n] = po

            o_sb = wgt.tile([1, N_LOC], f32, tag="o")

            def reduce_fin(n):     # += ones^T @ prod_7, copy out, DMA
                nc.tensor.matmul(pout[n][:], ones_sb[:], prod[(n, KH - 1)][:],
                                 start=False, stop=True)
                # psum -> sbuf on the (idle) Scalar engine: `copy` lives in
                # the same act table as relu, so no table reload.
                nc.scalar.activation(o_sb[:, n * 512:(n + 1) * 512],
                                     pout[n][:], AF.Copy)
                eng = nc.sync if n == 0 else nc.scalar
                eng.dma_start(out.ap()[:, n * 512:(n + 1) * 512],
                              o_sb[:, n * 512:(n + 1) * 512])

            # --- schedule. Chunk (0, m0) runs k0 then k1 the moment each
            # xt piece lands; mini-spins bridge the known DMA edges (xt0b,
            # w1m1, w1m2) so the PE streak never breaks -- a hole resets
            # the HAM activity window and costs far more than a spin. ---
            def minispin(k=1):
                for _ in range(k):
                    nc.tensor.matmul(wu[:, 0:256], wu_src[:, 0:128],
                                     wu_src[:, 0:256], start=True, stop=True)

            p1_mms(0, 0, 0, 2)
            minispin(N_WU_MID)
            p1_mms(0, 0, 2, KX)
            p1_act(0, 0)
            product(0, 0)
            for m in range(1, KH):
                if m <= 2:
                    minispin(1)
                phase1(0, m)
                product(0, m)
            m7 = KH - 1
            czero = nc.const_aps.aps[(f32, 0.0)]
            for m in range(KH):
                phase1(1, m) if m < m7 else p1_mms(1, m, 0, KX)
                if m == 3:
                    reduce_pre(0)
                elif m == 4:
                    reduce_fin(0)
                if m < m7:
                    product(1, m)
                else:
                    # tail: split the last relu across Scalar and DVE so
                    # the 0.7us activation no longer serializes the chain
                    nc.scalar.activation(
                        hT[m7][:, 512:768], psums[(1, m7)][:, 0:256],
                        AF.Relu, bias=b1_sb[:, m7:m7 + 1])
                    nc.vector.tensor_scalar(
                        hT[m7][:, 768:1024], psums[(1, m7)][:, 256:512],
                        b1_sb[:, m7:m7 + 1], czero, OP.add, OP.max)
                    p = pr.tile([128, 512], BF16, tag="pr1_7s",
                                name="pr1_7s")
                    nc.vector.tensor_tensor(
                        p[:, 256:512], hT[m7][:, 768:1024],
                        a_sb[m7][:, 768:1024], OP.mult)
                    nc.vector.tensor_tensor(
                        p[:, 0:256], hT[m7][:, 512:768],
                        a_sb[m7][:, 512:768], OP.mult)
                    prod[(1, m7)] = p
            nc.sync.dma_start(acc1D.ap(), acc[1][:])
            nc.scalar.dma_start(p7D.ap(), prod[(1, KH - 1)][:])

    nc.compile()
    return nc


def _get_nc():
    if "nc" not in _NC_CACHE:
        import concourse.bacc as bacc
        import concourse.mybir as mybir
        from concourse import tile
        _NC_CACHE["nc"] = _build(bacc.Bacc, mybir, tile)
    return _NC_CACHE["nc"]


def kernel(x_samples, y_idx, W1, b1, W2, b2):
    import ml_dtypes
    from concourse.bass_utils import run_bass_kernel_spmd

    bf16 = ml_dtypes.bfloat16
    x = np.ascontiguousarray(np.asarray(x_samples, dtype=np.float32))
    y = np.asarray(y_idx).astype(np.int64).reshape(-1)
    W1 = np.ascontiguousarray(np.asarray(W1, dtype=np.float32))
    b1 = np.asarray(b1, dtype=np.float32).reshape(-1)
    W2 = np.ascontiguousarray(np.asarray(W2, dtype=np.float32))
    b2 = np.asarray(b2, dtype=np.float32).reshape(-1)

    # global label histogram; fold normalizer-free negative term + bias
    c = np.bincount(y, minlength=Y_DIM).astype(np.float32)
    w2c = (W2 @ c) / np.float32(N)                                # [H]
    beta = np.float32(b2 @ c) / np.float32(N)
    g_full = (b2[y] - beta).astype(np.float32)                    # [N]

    # device layouts
    # w1_dev[m][p, k*128+c] = W1[k*128+p, m*128+c]
    w1_dev = np.ascontiguousarray(
        W1.reshape(KX, 128, KH, 128).transpose(2, 1, 0, 3)
        .reshape(KH, 128, KX * 128)).astype(bf16)
    w1_flat = np.ascontiguousarray(
        w1_dev.transpose(1, 0, 2).reshape(128, KH * 512))
    b1c = np.ascontiguousarray(b1.reshape(KH, 128).T)             # [128, 8]
    W2m = W2 - w2c[:, None]                                       # [H, Y]

    in_maps = []
    for mcore in range(N_CORES):
        sl = slice(mcore * N_LOC, (mcore + 1) * N_LOC)
        # xt_dev[n][p, k*512+r] = x[base + n*512+r, k*128+p]
        xt_dev = np.ascontiguousarray(
            x[sl].reshape(RG, 512, KX, 128).transpose(0, 3, 2, 1)
            .reshape(RG, 128, KX * 512)).astype(bf16)
        # a_dev[m][p, r] = W2m[m*128+p, y[base+r]]
        a_dev = W2m[:, y[sl]].reshape(KH, 128, N_LOC).astype(bf16)
        im = {
            "b1c": b1c,
            "w1a0": np.ascontiguousarray(w1_flat[:, 0:128]),
            "w1a1": np.ascontiguousarray(w1_flat[:, 128:512]),
            "xt0k0": np.ascontiguousarray(xt_dev[0][:, 0:512]),
            "xt0k1": np.ascontiguousarray(xt_dev[0][:, 512:1024]),
            "xt0b": np.ascontiguousarray(xt_dev[0][:, 1024:2048]),
            "xt1a": np.ascontiguousarray(xt_dev[1][:, 0:1024]),
            "xt1b": np.ascontiguousarray(xt_dev[1][:, 1024:2048]),
        }
        for m in range(1, KH):
            im[f"w1m{m}"] = np.ascontiguousarray(
                w1_flat[:, m * 512:(m + 1) * 512])
        for m in range(KH):
            for g in range(RG):
                im[f"am{m}g{g}"] = np.ascontiguousarray(
                    a_dev[m][:, g * 512:(g + 1) * 512])
        in_maps.append(im)

    nc = _get_nc()
    res = run_bass_kernel_spmd(nc, in_maps, core_ids=list(range(N_CORES)))
    parts = []
    for mc in range(N_CORES):
        r = res.results[mc]
        parts.append(np.asarray(r["out"]).reshape(-1)[0:512])
        # rgroup 1 ships its raw m0-6 product-accumulator (early, off the
        # tail) plus the last chunk's product separately; the partition
        # reduce happens here (identical math to the PSUM ones-matmul).
        parts.append(np.asarray(r["acc1"], dtype=np.float32).sum(axis=0)
                     + np.asarray(r["p7"], dtype=np.float32).sum(axis=0))
    dev = np.concatenate(parts)
    return (dev + g_full).astype(np.float32)


# revision 21
# speedup vs baseline: 1.0717x; 1.0451x over previous
"""Trainium2 Bass kernel for nn_CLUBCategorical (CLUB categorical loss).

Reference computation:
    h      = relu(x @ W1 + b1)              [N, H]
    logits = h @ W2 + b2                    [N, Y]
    logp   = log_softmax(logits, -1)        [N, Y]
    out[i] = logp[i, y_i] - mean_j logp[i, y_j]

The log-softmax normalizer cancels between the positive and negative
terms. With c[y] = histogram(y_idx) (global), w2c = (W2 @ c)/N:

    out[i] = h[i,:] @ (W2[:, y_i] - w2c) + (b2[y_i] - (b2 @ c)/N)
           = h[i,:] @ A[:, i] + g[i]

A is gathered on the HOST (it knows y), so the device only computes
phase-1 (h = relu(x@W1+b1), 64 matmuls) plus a cheap product-reduce:

    per m-chunk (128 hidden dims):  prod_m = hT_m * A_m      (DVE, bf16)
    acc = sum_{m<7} prod_m                                    (DVE chain)
    rgroup 0: out[1,512] = ones^T @ acc (+) ones^T @ prod_7   (2 matmuls,
                              PSUM-accumulated, mid-flight, then copied)
    rgroup 1: raw `acc1` ships EARLY plus the last chunk's product `p7`
              the moment it exists (its relu split across Scalar and
              DVE); the 128-partition reduce happens on the HOST.

g is added on the host during unsharding.

Hardware model (from neuron-profile traces of the previous builds):
- HAM clock gate: PE at 1.2GHz until ~3.4-5us of SUSTAINED matmul
  activity; an idle hole resets the streak (a 2.4us hole cost ~6us of
  cold execution in one build).  So the warmup spins must bridge the
  DMA lead-in exactly, with mini-spins over any known data gap.
- DMA: each transfer's completion semaphore fires ~1.3-2us after its
  last byte (HBM receipt round trip).  Queues (rings) round-robin at
  packet granularity, so concurrent streams split bandwidth; the
  critical prefix (xt0-k0 + w1-m0k0) is made SMALL and runs against
  only light contention.  The Tile scheduler keeps per-engine program
  order only among equally-ready instructions: a dep-blocked "gate"
  gets bypassed, so every deferred A config carries its OWN data dep
  (a dummy gpsimd write into that A tile, reading an xt1b-gated
  slice).  The sync ring's A-g0 configs are all ready at t=0 and so
  stay FIFO behind the xt stream without any gate.
- DMA configs block their issuing queue at ring depth 4; Scalar
  (which runs the 16 ReLUs from ~10.7us) carries exactly 4 input
  configs, issued before the first relu.

Sharding: data-parallel over N; each of 8 cores takes 1024 rows and the
full W1 plus its own gathered A block. No collectives.
"""

import numpy as np

N, X_DIM, Y_DIM, HIDDEN = 8192, 512, 512, 1024
N_CORES = 8
N_LOC = N // N_CORES          # 1024 rows per core
KX = X_DIM // 128             # 4  k-chunks (contraction), phase 1
KH = HIDDEN // 128            # 8  hidden chunks
RG = N_LOC // 512             # 2  row groups of 512

N_WU = 6                      # warmup matmuls (bridge until xt0k0 lands)
N_WU_MID = 3                  # 256-wide spins bridging xt0k1 -> xt0b

_NC_CACHE = {}


def _build(nc_cls, mybir, tile):
    mdt = mybir.dt
    f32 = mdt.float32
    F32R = mdt.float32r
    BF16 = mdt.bfloat16
    AF = mybir.ActivationFunctionType
    OP = mybir.AluOpType

    nc = nc_cls("TRN2", target_bir_lowering=False, debug=False,
                num_devices=N_CORES)

    # --- dram tensors (bf16 device layouts) ---
    # xt0 in three pieces (k0 | k1 | k23); xt1 in halves.
    xt0D = [nc.dram_tensor("xt0k0", [128, 512], BF16, kind="ExternalInput"),
            nc.dram_tensor("xt0k1", [128, 512], BF16, kind="ExternalInput"),
            nc.dram_tensor("xt0b", [128, 1024], BF16, kind="ExternalInput")]
    xt1D = [nc.dram_tensor(f"xt1{h}", [128, 1024], BF16,
                           kind="ExternalInput") for h in "ab"]
    # w1 split per m-chunk (m0 further split k0 | k123): each piece is
    # consumed by exactly one chunk's 4 matmuls, so the stream can land
    # in deadline order at ~128KB granularity.
    # layout w1[p, m*512 + k*128 + c] = W1[k*128+p, m*128+c]
    w1D = [nc.dram_tensor("w1a0", [128, 128], BF16, kind="ExternalInput"),
           nc.dram_tensor("w1a1", [128, 384], BF16, kind="ExternalInput")]
    w1D += [nc.dram_tensor(f"w1m{m}", [128, 512], BF16,
                           kind="ExternalInput") for m in range(1, KH)]
    # A stream, g-major packed: a_all[p, g*4096 + m*512 + r] =
    # W2m[m*128+p, y[g*512+r]].  Shipped as 7 mid-size transfers
    # (bigger transfers sustain more DMA bandwidth; the kernel is
    # DMA-throughput-bound at ~250-300 GB/s aggregate).
    aD = {k: nc.dram_tensor(f"a{k}", [128, sz], BF16, kind="ExternalInput")
          for k, sz in (("g0m0123", 2048), ("g0m45", 1024), ("g0m67", 1024),
                        ("g1m01", 1024), ("g1m23", 1024), ("g1m45", 1024),
                        ("g1m67", 1024))}
    # b1c: [128, 8]  b1c[p, m] = b1[m*128+p]
    b1D = nc.dram_tensor("b1c", [128, KH], f32, kind="ExternalInput")
    out = nc.dram_tensor("out", [1, N_LOC], f32, kind="ExternalOutput")
    acc1D = nc.dram_tensor("acc1", [128, 512], BF16, kind="ExternalOutput")
    p7D = nc.dram_tensor("p7", [128, 512], BF16, kind="ExternalOutput")

    with tile.TileContext(nc) as tc:
        with (
            tc.tile_pool(name="wgt", bufs=1) as wgt,
            tc.tile_pool(name="hp", bufs=1) as hp,
            tc.tile_pool(name="pr", bufs=1) as pr,
            tc.tile_pool(name="ps", bufs=1, space="PSUM") as ps,
        ):
            # --- on-chip constants.  wu_f memset runs on gpsimd FIRST so
            # the 512-wide PE spins can start right at body start. ---
            wu_f = wgt.tile([128, 512], f32, tag="wu")
            nc.gpsimd.memset(wu_f[:], 1.0)
            wu_src = wu_f.bitcast(F32R)
            ones_f = wgt.tile([128, 1], f32, tag="onesf")
            nc.vector.memset(ones_f[:], 1.0)
            ones_sb = wgt.tile([128, 1], BF16, tag="ones")
            nc.vector.tensor_copy(ones_sb[:], ones_f[:])

            b1_sb = wgt.tile([128, KH], f32, tag="b1")
            xt_sb = [wgt.tile([128, KX * 512], BF16, tag=f"xt{n}",
                              name=f"xt{n}") for n in range(RG)]
            w1_sb = wgt.tile([128, KH * 512], BF16, tag="w1")
            # one big A tile, g-major: [:, g*4096 + m*512 + r]
            a_all = wgt.tile([128, RG * KH * 512], BF16, tag="a_all")

            # --- DMA.  All transfers in GLOBAL DEADLINE ORDER, round-
            # robined across four rings.  Each ring's configs are all
            # ready at t=0, so per-ring priority order == program order
            # == ring FIFO: transfers drain in exactly this order, and
            # deferred transfers (A-g1) simply sit behind earlier ones on
            # the same deep ring -- no gates needed, no hoisting possible.
            # scalar and vector carry exactly 4 configs each (ring depth)
            # so their engines never block before the relus/products. ---
            def w1s(m):
                return w1_sb[:, m * 512:(m + 1) * 512]

            def aslc(name, off, sz):
                return (a_all[:, off:off + sz], aD[name].ap())

            nc.scalar.dma_start(w1_sb[:, 0:128], w1D[0].ap())        # w1a0
            nc.sync.dma_start(xt_sb[0][:, 0:512], xt0D[0].ap())      # xt0k0
            nc.gpsimd.dma_start(b1_sb[:], b1D.ap())                  # b1
            nc.scalar.dma_start(w1_sb[:, 128:512], w1D[1].ap())      # w1a1
            nc.sync.dma_start(xt_sb[0][:, 512:1024], xt0D[1].ap())   # xt0k1
            nc.gpsimd.dma_start(xt_sb[0][:, 1024:2048], xt0D[2].ap())  # xt0b
            nc.sync.dma_start(w1s(1), w1D[2].ap())                   # w1m1
            nc.scalar.dma_start(w1s(2), w1D[3].ap())                 # w1m2
            nc.sync.dma_start(w1s(3), w1D[4].ap())                   # w1m3
            nc.gpsimd.dma_start(w1s(4), w1D[5].ap())                 # w1m4
            nc.sync.dma_start(w1s(5), w1D[6].ap())                   # w1m5
            nc.gpsimd.dma_start(w1s(6), w1D[7].ap())                 # w1m6
            nc.gpsimd.dma_start(w1s(7), w1D[8].ap())                 # w1m7
            nc.scalar.dma_start(*aslc("g0m0123", 0, 2048))           # A
            nc.gpsimd.dma_start(xt_sb[1][:, 0:1024], xt1D[0].ap())   # xt1a
            nc.sync.dma_start(xt_sb[1][:, 1024:2048], xt1D[1].ap())  # xt1b
            nc.scalar.dma_start(*aslc("g0m67", 3072, 1024))
            nc.sync.dma_start(*aslc("g1m01", 4096, 1024))
            nc.gpsimd.dma_start(*aslc("g0m45", 2048, 1024))
            nc.scalar.dma_start(*aslc("g1m23", 4096 + 1024, 1024))
            nc.gpsimd.dma_start(*aslc("g1m45", 4096 + 2048, 1024))
            nc.sync.dma_start(*aslc("g1m67", 4096 + 3072, 1024))

            # --- PE warmup: tiny const-ap matmuls at body start, then
            # 512-wide spins until xt0k0 lands. ---
            cap = nc.const_aps.aps[(f32, 1.0)]
            wupre = ps.tile([1, 1], f32, tag="psum", bufs=6, name="wupre")
            for _ in range(8):
                nc.tensor.matmul(wupre[:], cap, cap, start=True, stop=True)
            wu = ps.tile([128, 512], f32, tag="psum", bufs=6, name="wu")
            for _ in range(N_WU):
                nc.tensor.matmul(wu[:], wu_src[:, 0:128], wu_src[:],
                                 start=True, stop=True)

            hT = [hp.tile([128, N_LOC], BF16, tag=f"h{m}", name=f"h{m}")
                  for m in range(KH)]
            prod = {}
            acc = {}

            def a_slice(n, m):
                off = n * 4096 + m * 512
                return a_all[:, off:off + 512]

            psums = {}

            def p1_mms(n, m, k0, k1):
                if (n, m) not in psums:
                    psums[(n, m)] = ps.tile([128, 512], f32, tag="psum",
                                            bufs=6, name=f"p1_{n}_{m}")
                psum = psums[(n, m)]
                for k in range(k0, k1):
                    nc.tensor.matmul(
                        psum[:],
                        w1_sb[:, m * 512 + k * 128: m * 512 + (k + 1) * 128],
                        xt_sb[n][:, k * 512:(k + 1) * 512],
                        start=(k == 0), stop=(k == KX - 1))

            def p1_act(n, m):
                nc.scalar.activation(
                    hT[m][:, n * 512:(n + 1) * 512], psums[(n, m)][:],
                    AF.Relu, bias=b1_sb[:, m:m + 1])

            def phase1(n, m):
                p1_mms(n, m, 0, KX)
                p1_act(n, m)

            def product(n, m):
                p = pr.tile([128, 512], BF16, tag=f"pr{n}_{m}",
                            name=f"pr{n}_{m}")
                nc.vector.tensor_tensor(
                    p[:], hT[m][:, n * 512:(n + 1) * 512],
                    a_slice(n, m), OP.mult)
                prod[(n, m)] = p
                if m == 0:
                    acc[n] = p
                elif m < KH - 1:   # m7 handled separately (matmul / p7D)
                    a2 = pr.tile([128, 512], BF16, tag=f"ac{n}_{m}",
                                 name=f"ac{n}_{m}")
                    nc.vector.tensor_tensor(a2[:], acc[n][:], p[:], OP.add)
                    acc[n] = a2

            pout = {}

            def reduce_pre(n):     # ones^T @ acc(m0..m6) -> pout[n]
                po = ps.tile([1, 512], f32, tag=f"po{n}", bufs=1,
                             name=f"po{n}")
                nc.tensor.matmul(po[:], ones_sb[:], acc[n][:],
                                 start=True, stop=False)
                pout[n] = po

            o_sb = wgt.tile([1, N_LOC], f32, tag="o")

            def reduce_fin(n):     # += ones^T @ prod_7, copy out, DMA
                nc.tensor.matmul(pout[n][:], ones_sb[:], prod[(n, KH - 1)][:],
                                 start=False, stop=True)
                # psum -> sbuf on the (idle) Scalar engine: `copy` lives in
                # the same act table as relu, so no table reload.
                nc.scalar.activation(o_sb[:, n * 512:(n + 1) * 512],
                                     pout[n][:], AF.Copy)
                eng = nc.sync if n == 0 else nc.scalar
                eng.dma_start(out.ap()[:, n * 512:(n + 1) * 512],
                              o_sb[:, n * 512:(n + 1) * 512])

            # --- schedule. Chunk (0, m0) runs k0 then k1 the moment each
            # xt piece lands; mini-spins bridge the known DMA edges (xt0b,
            # w1m1, w1m2) so the PE streak never breaks -- a hole resets
            # the HAM activity window and costs far more than a spin. ---
            def minispin(k=1):
                for _ in range(k):
                    nc.tensor.matmul(wu[:, 0:256], wu_src[:, 0:128],
                                     wu_src[:, 0:256], start=True, stop=True)

            p1_mms(0, 0, 0, 2)
            minispin(N_WU_MID)
            p1_mms(0, 0, 2, KX)
            p1_act(0, 0)
            product(0, 0)
            for m in range(1, KH):
                if m <= 2:
                    minispin(1)
                phase1(0, m)
                product(0, m)
            m7 = KH - 1
            czero = nc.const_aps.aps[(f32, 0.0)]
            for m in range(KH):
                phase1(1, m) if m < m7 else p1_mms(1, m, 0, KX)
                if m == 6:
                    reduce_pre(0)
                elif m == m7:
                    reduce_fin(0)
                if m < m7:
                    product(1, m)
                else:
                    # tail: split the last relu across Scalar and DVE so
                    # the 0.7us activation no longer serializes the chain
                    nc.scalar.activation(
                        hT[m7][:, 512:768], psums[(1, m7)][:, 0:256],
                        AF.Relu, bias=b1_sb[:, m7:m7 + 1])
                    nc.vector.tensor_scalar(
                        hT[m7][:, 768:1024], psums[(1, m7)][:, 256:512],
                        b1_sb[:, m7:m7 + 1], czero, OP.add, OP.max)
                    p = pr.tile([128, 512], BF16, tag="pr1_7s",
                                name="pr1_7s")
                    a17 = a_slice(1, m7)
                    nc.vector.tensor_tensor(
                        p[:, 256:512], hT[m7][:, 768:1024],
                        a17[:, 256:512], OP.mult)
                    nc.vector.tensor_tensor(
                        p[:, 0:256], hT[m7][:, 512:768],
                        a17[:, 0:256], OP.mult)
                    prod[(1, m7)] = p
            nc.sync.dma_start(acc1D.ap(), acc[1][:])
            nc.scalar.dma_start(p7D.ap(), prod[(1, KH - 1)][:])

    nc.compile()
    return nc


def _get_nc():
    if "nc" not in _NC_CACHE:
        import concourse.bacc as bacc
        import concourse.mybir as mybir
        from concourse import tile
        _NC_CACHE["nc"] = _build(bacc.Bacc, mybir, tile)
    return _NC_CACHE["nc"]


def kernel(x_samples, y_idx, W1, b1, W2, b2):
    import ml_dtypes
    from concourse.bass_utils import run_bass_kernel_spmd

    bf16 = ml_dtypes.bfloat16
    x = np.ascontiguousarray(np.asarray(x_samples, dtype=np.float32))
    y = np.asarray(y_idx).astype(np.int64).reshape(-1)
    W1 = np.ascontiguousarray(np.asarray(W1, dtype=np.float32))
    b1 = np.asarray(b1, dtype=np.float32).reshape(-1)
    W2 = np.ascontiguousarray(np.asarray(W2, dtype=np.float32))
    b2 = np.asarray(b2, dtype=np.float32).reshape(-1)

    # global label histogram; fold normalizer-free negative term + bias
    c = np.bincount(y, minlength=Y_DIM).astype(np.float32)
    w2c = (W2 @ c) / np.float32(N)                                # [H]
    beta = np.float32(b2 @ c) / np.float32(N)
    g_full = (b2[y] - beta).astype(np.float32)                    # [N]

    # device layouts
    # w1_dev[m][p, k*128+c] = W1[k*128+p, m*128+c]
    w1_dev = np.ascontiguousarray(
        W1.reshape(KX, 128, KH, 128).transpose(2, 1, 0, 3)
        .reshape(KH, 128, KX * 128)).astype(bf16)
    w1_flat = np.ascontiguousarray(
        w1_dev.transpose(1, 0, 2).reshape(128, KH * 512))
    b1c = np.ascontiguousarray(b1.reshape(KH, 128).T)             # [128, 8]
    W2m = W2 - w2c[:, None]                                       # [H, Y]

    in_maps = []
    for mcore in range(N_CORES):
        sl = slice(mcore * N_LOC, (mcore + 1) * N_LOC)
        # xt_dev[n][p, k*512+r] = x[base + n*512+r, k*128+p]
        xt_dev = np.ascontiguousarray(
            x[sl].reshape(RG, 512, KX, 128).transpose(0, 3, 2, 1)
            .reshape(RG, 128, KX * 512)).astype(bf16)
        # a_dev[m][p, r] = W2m[m*128+p, y[base+r]]; packed g-major:
        # a_gm[g][p, m*512+r] = a_dev[m][p, g*512+r]
        a_dev = W2m[:, y[sl]].reshape(KH, 128, N_LOC).astype(bf16)
        a_gm = np.ascontiguousarray(
            a_dev.reshape(KH, 128, RG, 512).transpose(2, 1, 0, 3)
            .reshape(RG, 128, KH * 512))
        im = {
            "b1c": b1c,
            "w1a0": np.ascontiguousarray(w1_flat[:, 0:128]),
            "w1a1": np.ascontiguousarray(w1_flat[:, 128:512]),
            "xt0k0": np.ascontiguousarray(xt_dev[0][:, 0:512]),
            "xt0k1": np.ascontiguousarray(xt_dev[0][:, 512:1024]),
            "xt0b": np.ascontiguousarray(xt_dev[0][:, 1024:2048]),
            "xt1a": np.ascontiguousarray(xt_dev[1][:, 0:1024]),
            "xt1b": np.ascontiguousarray(xt_dev[1][:, 1024:2048]),
        }
        for m in range(1, KH):
            im[f"w1m{m}"] = np.ascontiguousarray(
                w1_flat[:, m * 512:(m + 1) * 512])
        for name, g, c0, c1 in (("g0m0123", 0, 0, 2048),
                                ("g0m45", 0, 2048, 3072),
                                ("g0m67", 0, 3072, 4096),
                                ("g1m01", 1, 0, 1024),
                                ("g1m23", 1, 1024, 2048),
                                ("g1m45", 1, 2048, 3072),
                                ("g1m67", 1, 3072, 4096)):
            im[f"a{name}"] = np.ascontiguousarray(a_gm[g][:, c0:c1])
        in_maps.append(im)

    nc = _get_nc()
    res = run_bass_kernel_spmd(nc, in_maps, core_ids=list(range(N_CORES)))
    parts = []
    for mc in range(N_CORES):
        r = res.results[mc]
        parts.append(np.asarray(r["out"]).reshape(-1)[0:512])
        # rgroup 1 ships its raw m0-6 product-accumulator (early, off the
        # tail) plus the last chunk's product separately; the partition
        # reduce happens here (identical math to the PSUM ones-matmul).
        parts.append(np.asarray(r["acc1"], dtype=np.float32).sum(axis=0)
                     + np.asarray(r["p7"], dtype=np.float32).sum(axis=0))
    dev = np.concatenate(parts)
    return (dev + g_full).astype(np.float32)
